# revision 1
# baseline (speedup 1.0000x reference)
"""ClusterGCNConv 2-layer encoder (N=100000, E=640000, 128->128->16) on 8
TRN2 NeuronCores. Self-contained: kernel(**inputs) -> full [100000,16] output.

Strategy: nodes sharded 8 ways by contiguous target blocks; edges partitioned
by target. All-bf16 data path: per-256(phase1)/128(phase2)-target tiles the
kernel dma_gathers bf16 source rows (256B elements, int16-bucketed indices,
2048-idx windows on 4 SWDGE queues), builds 0/1 onehot matrices on DVE
(batched is_equal on broadcast APs), and scatter-adds via bf16 PE matmuls into
f32 PSUM. Edge weights w=deg_inv[col] depend only on the target, so no
per-edge scaling: deg_inv is applied once post-aggregation (DVE mult with a
host-precomputed broadcast table in phase 1; fused into the final combine in
phase 2). Self loops are applied via identity matmuls of raw x_own before the
deg_inv scale. y2 = h @ W_out2 ([N,16] bf16) is AllGathered between layers so
layer-2 messages gather 256B blocks of 8 packed rows from a [N/8,128] view,
with an is_equal group-select mask on DVE and a tensor_reduce group-sum.
"""


import sys

sys.path.insert(0, "/opt/trn_rl_repo")

from contextlib import ExitStack  # noqa: E402

import ml_dtypes  # noqa: E402
import numpy as np  # noqa: E402

import concourse.bass as bass  # noqa: E402
import concourse.tile as tile  # noqa: E402
from concourse import bacc, mybir  # noqa: E402
from concourse.masks import make_identity  # noqa: E402

F32 = mybir.dt.float32
BF16 = mybir.dt.bfloat16
I16 = mybir.dt.int16
AOT = mybir.AluOpType
BFNP = ml_dtypes.bfloat16

BUCKET = 32768
WIN = 1024  # gather slots per dma_gather instruction
NQ = 4  # SWDGE queues
STW = 256  # phase-1 super-tile width (targets)
LOOKAHEAD = 32  # chunks of gather prefetch


def _round_up(a, m):
    return (a + m - 1) // m * m


def _within_group(keys):
    diff = np.concatenate([[True], keys[1:] != keys[:-1]])
    grp_start = np.flatnonzero(diff)
    grp_id = np.cumsum(diff) - 1
    return np.arange(len(keys)) - grp_start[grp_id]


def _instances(seg_start, seg_len, icol0):
    out = []
    icol = icol0
    for s0, ln in zip(seg_start, seg_len):
        lst = []
        if ln > 0:
            for C in range(int(s0) // 128, (int(s0) + int(ln) - 1) // 128 + 1):
                lst.append((C, icol))
                icol += 1
        out.append(lst)
    return out, icol


def wrap16(arr):
    a = arr.reshape(-1, 16).T
    return np.ascontiguousarray(np.tile(a, (8, 1)))


def preprocess(x, edge_index, n_cores):
    N = x.shape[0]
    row = np.asarray(edge_index[0], dtype=np.int64)
    col = np.asarray(edge_index[1], dtype=np.int64)
    not_self = row != col
    deg = np.bincount(col[not_self], minlength=N).astype(np.float64) + 1.0
    di = (1.0 / np.maximum(deg, 1.0)).astype(np.float32)

    r_all = row[not_self]
    c_all = col[not_self]

    assert N % n_cores == 0
    shard = N // n_cores
    n_tiles = (shard + 127) // 128
    n_st = (shard + STW - 1) // STW
    n_buckets = (N + BUCKET - 1) // BUCKET

    core_of = c_all // shard
    st_of = (c_all % shard) // STW
    t2_of = (c_all % shard) // 128
    buck_of = r_all // BUCKET

    o1 = np.lexsort((buck_of, st_of, core_of))
    o2 = np.lexsort((t2_of, core_of))

    # phase-1 per-bucket streams of (st) segments
    cnt1 = np.zeros((n_cores, n_st, n_buckets), dtype=np.int64)
    np.add.at(cnt1, (core_of, st_of, buck_of), 1)
    seg_len_sb = cnt1.max(axis=0)  # [n_st, n_buckets]
    seg_start_sb = np.zeros_like(seg_len_sb)
    L1 = np.zeros(n_buckets, dtype=np.int64)
    for b in range(n_buckets):
        s = 0
        for st in range(n_st):
            seg_start_sb[st, b] = s
            s += seg_len_sb[st, b]
        L1[b] = _round_up(max(int(s), 128), 128)

    cnt2 = np.zeros((n_cores, n_tiles), dtype=np.int64)
    np.add.at(cnt2, (core_of, t2_of), 1)
    seg2_len = cnt2.max(axis=0)
    seg2_start = np.concatenate([[0], np.cumsum(seg2_len)[:-1]])
    L2 = _round_up(max(int(seg2_len.sum()), 128), 128)

    inst1 = [[] for _ in range(n_st)]
    icol = 0
    for st in range(n_st):
        for b in range(n_buckets):
            ln = int(seg_len_sb[st, b])
            if ln == 0:
                continue
            s0 = int(seg_start_sb[st, b])
            for C in range(s0 // 128, (s0 + ln - 1) // 128 + 1):
                inst1[st].append((b, C, icol))
                icol += 1
    NI1 = _round_up(icol, 2)

    inst2, icol2 = _instances(seg2_start, seg2_len, 0)
    NI2 = _round_up(icol2, 4)
    NC2 = L2 // 128

    off16 = np.zeros(n_buckets + 1, dtype=np.int64)
    for b in range(n_buckets):
        off16[b + 1] = off16[b] + L1[b] // 16

    half_a = (n_st // 2) * STW

    per_core = []
    for c in range(n_cores):
        m1 = o1[core_of[o1] == c]
        r1, c1 = r_all[m1], c_all[m1]
        st1, b1v = st_of[m1], buck_of[m1]
        within1 = _within_group(st1 * n_buckets + b1v)
        slot1 = seg_start_sb[st1, b1v] + within1
        cl1 = (c1 - (c * shard + st1 * STW)).astype(np.float32)
        loc1 = (r1 - b1v * BUCKET).astype(np.int16)

        idx1 = [np.zeros(int(L1[b]), dtype=np.int16) for b in range(n_buckets)]
        colv1 = [np.full(int(L1[b]), -1.0, dtype=np.float32) for b in range(n_buckets)]
        for b in range(n_buckets):
            m = b1v == b
            idx1[b][slot1[m]] = loc1[m]
            colv1[b][slot1[m]] = cl1[m]

        colf1 = np.full((128, NI1), -1.0, dtype=np.float32)
        for st in range(n_st):
            for b, C, ic in inst1[st]:
                s0 = int(seg_start_sb[st, b])
                ln = int(seg_len_sb[st, b])
                lo = C * 128
                a0, a1 = max(lo, s0), min(lo + 128, s0 + ln)
                if a0 < a1:
                    colf1[a0 - lo : a1 - lo, ic] = colv1[b][a0:a1]

        m2 = o2[core_of[o2] == c]
        r2, c2 = r_all[m2], c_all[m2]
        t2 = t2_of[m2]
        within2 = _within_group(t2)
        slot2 = seg2_start[t2] + within2
        idx2 = np.zeros(int(L2), dtype=np.int16)
        colv2 = np.full(int(L2), -1.0, dtype=np.float32)
        qv2 = np.zeros(int(L2), dtype=np.float32)
        l2loc = r2 % shard
        c2core = r2 // shard
        pos = np.where(
            l2loc < half_a,
            c2core * half_a + l2loc,
            n_cores * half_a + c2core * (shard - half_a) + (l2loc - half_a),
        )
        idx2[slot2] = (pos >> 3).astype(np.int16)
        colv2[slot2] = (c2 - (c * shard + t2 * 128)).astype(np.float32)
        qv2[slot2] = (pos & 7).astype(np.float32)

        colf2 = np.full((128, NI2), -1.0, dtype=np.float32)
        for lst, s0, ln in zip(inst2, seg2_start, seg2_len):
            s0, ln = int(s0), int(ln)
            for C, ic in lst:
                lo = C * 128
                a0, a1 = max(lo, s0), min(lo + 128, s0 + ln)
                if a0 < a1:
                    colf2[a0 - lo : a1 - lo, ic] = colv2[a0:a1]
        qf2 = np.ascontiguousarray(qv2.reshape(NC2, 128).T)

        di_c = di[c * shard : (c + 1) * shard]
        npad = n_tiles * 128 - shard
        di_pad = np.pad(di_c, (0, npad))
        di_all = np.ascontiguousarray(di_pad.reshape(n_tiles, 128).T)
        # deg_inv broadcast along partitions, indexed by target in the free dim
        di_bc = np.ascontiguousarray(
            np.broadcast_to(di_pad[None, :], (128, n_tiles * 128))
        )

        idx16_1 = np.concatenate([wrap16(idx1[b]) for b in range(n_buckets)], axis=1)
        idx16_2 = wrap16(idx2)

        per_core.append(
            dict(
                idx16_1=idx16_1,
                idx16_2=idx16_2,
                colf1=colf1.astype(BFNP),
                colf2=colf2.astype(BFNP),
                qf2=qf2.astype(BFNP),
                di_all=di_all,
                di_bc=di_bc,
            )
        )

    return dict(
        per_core=per_core,
        N=N,
        shard=shard,
        n_tiles=n_tiles,
        n_st=n_st,
        n_buckets=n_buckets,
        L1=[int(v) for v in L1],
        L2=int(L2),
        off16=[int(v) for v in off16],
        seg_start_sb=seg_start_sb,
        seg_len_sb=seg_len_sb,
        seg2_start=seg2_start,
        seg2_len=seg2_len,
        inst1=inst1,
        inst2=inst2,
        NI1=NI1,
        NI2=NI2,
        NC2=int(NC2),
        half_a=half_a,
    )


def build_kernel(nc, tc, meta, n_cores):
    N = meta["N"]
    shard = meta["shard"]
    n_tiles = meta["n_tiles"]
    n_st = meta["n_st"]
    n_buckets = meta["n_buckets"]
    L1 = meta["L1"]
    L2 = meta["L2"]
    off16 = meta["off16"]
    inst1 = meta["inst1"]
    inst2 = meta["inst2"]
    NI1, NI2, NC2 = meta["NI1"], meta["NI2"], meta["NC2"]
    half_a = meta["half_a"]

    x_bf = nc.dram_tensor("x_bf", [N, 128], BF16, kind="ExternalInput").ap()
    x_own = nc.dram_tensor("x_own", [n_tiles * 128, 128], BF16, kind="ExternalInput").ap()
    x_ownT = nc.dram_tensor("x_ownT", [128, n_tiles * 128], BF16, kind="ExternalInput").ap()
    di_bc = nc.dram_tensor("di_bc", [128, n_tiles * 128], F32, kind="ExternalInput").ap()
    idx16_1 = nc.dram_tensor("idx16_1", [128, off16[-1]], I16, kind="ExternalInput").ap()
    idx16_2 = nc.dram_tensor("idx16_2", [128, L2 // 16], I16, kind="ExternalInput").ap()
    colf1 = nc.dram_tensor("colf1", [128, NI1], BF16, kind="ExternalInput").ap()
    colf2 = nc.dram_tensor("colf2", [128, NI2], BF16, kind="ExternalInput").ap()
    qf2 = nc.dram_tensor("qf2", [128, NC2], BF16, kind="ExternalInput").ap()
    di_all = nc.dram_tensor("di_all", [128, n_tiles], F32, kind="ExternalInput").ap()
    wo1 = nc.dram_tensor("wo1", [128, 128], BF16, kind="ExternalInput").ap()
    wr1 = nc.dram_tensor("wr1", [128, 128], BF16, kind="ExternalInput").ap()
    wo2 = nc.dram_tensor("wo2", [128, 16], BF16, kind="ExternalInput").ap()
    wr2 = nc.dram_tensor("wr2", [128, 16], BF16, kind="ExternalInput").ap()
    b1 = nc.dram_tensor("b1", [128, 1], F32, kind="ExternalInput").ap()
    b2 = nc.dram_tensor("b2", [1, 16], BF16, kind="ExternalInput").ap()
    out = nc.dram_tensor("out", [shard, 16], F32, kind="ExternalOutput").ap()

    ctx = ExitStack()
    const = ctx.enter_context(tc.tile_pool(name="const", bufs=1))
    dram = ctx.enter_context(tc.tile_pool(name="dram", bufs=1, space="DRAM"))

    iotaP1 = const.tile([128, 2 * STW], BF16, tag="iotaP1")
    nc.gpsimd.iota(
        iotaP1[:], pattern=[[0, 2], [1, STW]], base=0, channel_multiplier=0,
        allow_small_or_imprecise_dtypes=True,
    )
    iotaP2 = const.tile([128, 512], BF16, tag="iotaP2")
    nc.gpsimd.iota(
        iotaP2[:], pattern=[[0, 4], [1, 128]], base=0, channel_multiplier=0,
        allow_small_or_imprecise_dtypes=True,
    )
    # (col//16)%8 over 2048 columns: group-select pattern for 8 packed rows
    iotaD16 = const.tile([128, WIN], BF16, tag="iotaD16")
    nc.gpsimd.iota(
        iotaD16[:], pattern=[[0, WIN // 128], [1, 8], [0, 16]], base=0,
        channel_multiplier=0, allow_small_or_imprecise_dtypes=True,
    )
    ident = const.tile([128, 128], BF16, tag="ident")
    make_identity(nc, ident[:])
    ones1 = const.tile([1, 128], BF16, tag="ones1")
    nc.vector.memset(ones1[:], 1.0)

    def load_const(name, ap, shape, dtype=F32):
        t = const.tile(shape, dtype, tag=name)
        nc.sync.dma_start(t[:], ap)
        return t

    idx16_1_sb = load_const("idx16_1", idx16_1, [128, off16[-1]], I16)
    idx16_2_sb = load_const("idx16_2", idx16_2, [128, L2 // 16], I16)
    colf1_sb = load_const("colf1", colf1, [128, NI1], BF16)
    colf2_sb = load_const("colf2", colf2, [128, NI2], BF16)
    qf2_sb = load_const("qf2", qf2, [128, NC2], BF16)
    di_sb = load_const("di_all", di_all, [128, n_tiles])
    wo1_sb = load_const("wo1", wo1, [128, 128], BF16)
    wr1_sb = load_const("wr1", wr1, [128, 128], BF16)
    wo2_sb = load_const("wo2", wo2, [128, 16], BF16)
    wr2_sb = load_const("wr2", wr2, [128, 16], BF16)
    b1_sb = load_const("b1", b1, [128, 1])
    b2_sb = load_const("b2", b2, [1, 16], BF16)

    r2b_all = const.tile([128, n_tiles * 16], F32, tag="r2b")
    y2_all = const.tile([128, n_tiles * 16], BF16, tag="y2a")

    y2_shard = dram.tile([shard, 16], BF16)
    y2_full = dram.tile([N, 16], BF16)

    relu = mybir.ActivationFunctionType.Relu
    qcount = [0]

    def gather(pool, tag, idxs_sb, col16, n_idx, src_ap, elem):
        gt = pool.tile([128, (WIN // 128) * elem], BF16, tag=tag)
        nc.gpsimd.dma_gather(
            out_ap=gt[:, : (n_idx // 128) * elem].rearrange(
                "p (s c) -> p s c", c=elem
            ),
            in_ap=src_ap,
            idxs_ap=idxs_sb[:, col16 : col16 + n_idx // 16],
            num_idxs=n_idx,
            num_idxs_reg=n_idx,
            elem_size=elem,
            queue_num=qcount[0] % NQ,
        )
        qcount[0] += 1
        return gt

    # ---------------- phase 1 ----------------
    seg_start_sb, seg_len_sb = meta["seg_start_sb"], meta["seg_len_sb"]
    with tc.tile_pool(name="p1", bufs=3) as p1, tc.tile_pool(
        name="ps1", bufs=2, space="PSUM"
    ) as ps1, tc.tile_pool(name="g1", bufs=3) as gp1:
        bucket_ap = []
        for b in range(n_buckets):
            lo = b * BUCKET
            hi = min(N, lo + BUCKET)
            bucket_ap.append(x_bf[lo:hi, :])
        win_tiles = [{} for _ in range(n_buckets)]
        next_win = [0] * n_buckets
        oh1_tiles = {}

        def pull_windows(b, upto_chunk):
            while (
                next_win[b] * WIN < (upto_chunk + 1) * 128
                and next_win[b] * WIN < L1[b]
            ):
                w = next_win[b]
                start = w * WIN
                n_idx = min(WIN, L1[b] - start)
                gt = gather(
                    gp1, f"g1b{b}", idx16_1_sb, off16[b] + start // 16,
                    n_idx, bucket_ap[b], 128,
                )
                win_tiles[b][w] = gt
                next_win[b] += 1

        def get_oh1(ic):
            g = ic // 2
            if g not in oh1_tiles:
                oh = p1.tile([128, 2 * STW], BF16, tag="oh")
                nc.vector.tensor_tensor(
                    out=oh[:].rearrange("p (a b) -> p a b", a=2),
                    in0=iotaP1[:].rearrange("p (a b) -> p a b", a=2),
                    in1=colf1_sb[:, 2 * g : 2 * g + 2].to_broadcast([128, 2, STW]),
                    op=AOT.is_equal,
                )
                oh1_tiles[g] = oh
            return oh1_tiles[g]

        for st in range(n_st):
            for b in range(n_buckets):
                ln = int(seg_len_sb[st, b])
                if ln:
                    s0 = int(seg_start_sb[st, b])
                    pull_windows(b, (s0 + ln - 1) // 128 + LOOKAHEAD)

            insts = inst1[st]
            nh = min(STW // 128, n_tiles - st * (STW // 128))
            # raw x_own tiles for the self-loop / root paths
            xo_tiles = []
            for h in range(nh):
                t = st * (STW // 128) + h
                x_own_sb = p1.tile([128, 128], BF16, tag="xo")
                nc.sync.dma_start(x_own_sb[:], x_own[t * 128 : (t + 1) * 128, :])
                xo_tiles.append(x_own_sb)
            xT_st = p1.tile([128, STW], BF16, tag="xT")
            nc.sync.dma_start(
                xT_st[:, : nh * 128], x_ownT[:, st * STW : st * STW + nh * 128]
            )
            di_st = p1.tile([128, STW], F32, tag="dist")
            nc.sync.dma_start(
                di_st[:, : nh * 128], di_bc[:, st * STW : st * STW + nh * 128]
            )

            aggT = ps1.tile([128, STW], F32, tag="agg")
            for k, (b, C, ic) in enumerate(insts):
                oh = get_oh1(ic)
                gt = win_tiles[b][C // (WIN // 128)]
                sl = C % (WIN // 128)
                nc.tensor.matmul(
                    aggT[:],
                    lhsT=gt[:, sl * 128 : (sl + 1) * 128],
                    rhs=oh[:, (ic % 2) * STW : (ic % 2 + 1) * STW],
                    start=(k == 0),
                    stop=False,
                )
            for h in range(nh):
                nc.tensor.matmul(
                    aggT[:, h * 128 : (h + 1) * 128],
                    lhsT=xo_tiles[h][:],
                    rhs=ident[:],
                    start=(len(insts) == 0),
                    stop=(h == nh - 1),
                )
            # deg_inv applied once per target column, psum->sbuf fused
            aggT_sb = p1.tile([128, STW], BF16, tag="aggsb")
            nc.vector.tensor_tensor(
                out=aggT_sb[:], in0=aggT[:], in1=di_st[:], op=AOT.mult
            )

            for h in range(nh):
                t = st * (STW // 128) + h
                pt = min(128, shard - t * 128)

                pT_ps = ps1.tile([128, 128], F32, tag="p1")
                nc.tensor.matmul(
                    pT_ps[:], lhsT=wo1_sb[:],
                    rhs=aggT_sb[:, h * 128 : (h + 1) * 128],
                    start=True, stop=False,
                )
                nc.tensor.matmul(
                    pT_ps[:], lhsT=wr1_sb[:],
                    rhs=xT_st[:, h * 128 : (h + 1) * 128],
                    start=False, stop=True,
                )
                hT_sb = p1.tile([128, 128], BF16, tag="ht")
                nc.scalar.activation(hT_sb[:], pT_ps[:], relu, bias=b1_sb[:, :1])

                yr_ps = ps1.tile([128, 32], F32, tag="small")
                nc.tensor.matmul(yr_ps[:, 0:16], lhsT=hT_sb[:], rhs=wo2_sb[:], start=True, stop=True)
                nc.tensor.matmul(yr_ps[:, 16:32], lhsT=hT_sb[:], rhs=wr2_sb[:], start=True, stop=False)
                nc.tensor.matmul(yr_ps[:, 16:32], lhsT=ones1[:], rhs=b2_sb[:], start=False, stop=True)

                nc.scalar.mul(y2_all[:, t * 16 : (t + 1) * 16], yr_ps[:, 0:16], 1.0)
                nc.sync.dma_start(
                    y2_shard[t * 128 : t * 128 + pt, :],
                    y2_all[:pt, t * 16 : (t + 1) * 16],
                )
                nc.scalar.mul(r2b_all[:, t * 16 : (t + 1) * 16], yr_ps[:, 16:32], 1.0)

            if half_a and (st + 1) * STW == half_a:
                nc.gpsimd.collective_compute(
                    "AllGather", AOT.bypass,
                    replica_groups=[list(range(n_cores))],
                    ins=[y2_shard[0:half_a, :]],
                    outs=[y2_full[0 : half_a * n_cores, :]],
                )

    # ---------------- allgather (second half / fallback) ----------------
    if half_a:
        nc.gpsimd.collective_compute(
            "AllGather", AOT.bypass,
            replica_groups=[list(range(n_cores))],
            ins=[y2_shard[half_a:shard, :]],
            outs=[y2_full[half_a * n_cores : N, :]],
        )
    else:
        nc.gpsimd.collective_compute(
            "AllGather", AOT.bypass,
            replica_groups=[list(range(n_cores))],
            ins=[y2_shard[:]],
            outs=[y2_full[:]],
        )

    # ---------------- phase 2 ----------------
    seg2_start, seg2_len = meta["seg2_start"], meta["seg2_len"]
    y2p = y2_full[:].rearrange("(a b) c -> a (b c)", b=8)  # [N/8, 128] bf16
    with tc.tile_pool(name="p2", bufs=4) as p2, tc.tile_pool(
        name="ps2", bufs=2, space="PSUM"
    ) as ps2, tc.tile_pool(name="g2", bufs=4) as gp2:
        y2m_tiles = {}
        next2 = [0]
        oh2_tiles = {}

        def pull2(upto_chunk):
            while next2[0] * WIN < (upto_chunk + 1) * 128 and next2[0] * WIN < L2:
                w = next2[0]
                start = w * WIN
                n_idx = min(WIN, L2 - start)
                gt = gather(gp2, "g2", idx16_2_sb, start // 16, n_idx, y2p, 128)
                nch = n_idx // 128
                # group-select mask: keep the 16 lanes of this slot's q
                msk = p2.tile([128, WIN], BF16, tag="msk")
                m3 = msk[:, : nch * 128].rearrange("p (a b) -> p a b", a=nch)
                nc.vector.tensor_tensor(
                    out=m3,
                    in0=iotaD16[:, : nch * 128].rearrange("p (a b) -> p a b", a=nch),
                    in1=qf2_sb[:, w * (WIN // 128) : w * (WIN // 128) + nch].to_broadcast(
                        [128, nch, 128]
                    ),
                    op=AOT.is_equal,
                )
                y2m = p2.tile([128, WIN], BF16, tag="y2m")
                nc.vector.tensor_tensor(
                    out=y2m[:, : nch * 128], in0=gt[:, : nch * 128],
                    in1=msk[:, : nch * 128], op=AOT.mult,
                )
                y2m_tiles[w] = y2m
                next2[0] += 1

        def get_oh2(ic):
            g = ic // 4
            if g not in oh2_tiles:
                oh = p2.tile([128, 512], BF16, tag="oh2")
                nc.vector.tensor_tensor(
                    out=oh[:].rearrange("p (a b) -> p a b", a=4),
                    in0=iotaP2[:].rearrange("p (a b) -> p a b", a=4),
                    in1=colf2_sb[:, 4 * g : 4 * g + 4].to_broadcast([128, 4, 128]),
                    op=AOT.is_equal,
                )
                oh2_tiles[g] = oh
            return oh2_tiles[g]

        for t in range(n_tiles):
            pt = min(128, shard - t * 128)
            ln = int(seg2_len[t])
            s0 = int(seg2_start[t])
            pull2(min((s0 + ln - 1) // 128 + LOOKAHEAD, L2 // 128 - 1))

            insts = inst2[t]
            o128 = ps2.tile([128, 128], F32, tag="o128")
            for k, (C, ic) in enumerate(insts):
                oh = get_oh2(ic)
                w = C // (WIN // 128)
                sl = C % (WIN // 128)
                nc.tensor.matmul(
                    o128[:],
                    lhsT=oh[:, (ic % 4) * 128 : (ic % 4 + 1) * 128],
                    rhs=y2m_tiles[w][:, sl * 128 : (sl + 1) * 128],
                    start=(k == 0),
                    stop=(k == len(insts) - 1),
                )
            # sum the 8 packed groups, add self y2, scale by deg_inv, add root
            o_sb = p2.tile([128, 128], F32, tag="osb")
            nc.scalar.mul(o_sb[:], o128[:], 1.0)
            s64 = p2.tile([128, 64], F32, tag="s64")
            nc.vector.tensor_add(s64[:], o_sb[:, 0:64], o_sb[:, 64:128])
            nc.vector.tensor_add(s64[:, 0:32], s64[:, 0:32], s64[:, 32:64])
            nc.vector.tensor_add(s64[:, 0:16], s64[:, 0:16], s64[:, 16:32])
            oa = p2.tile([128, 16], F32, tag="oa")
            nc.vector.tensor_add(oa[:], s64[:, 0:16], y2_all[:, t * 16 : (t + 1) * 16])
            nc.vector.tensor_tensor(
                out=oa[:], in0=oa[:],
                in1=di_sb[:, t : t + 1].to_broadcast([128, 16]),
                op=AOT.mult,
            )
            nc.vector.tensor_add(oa[:], oa[:], r2b_all[:, t * 16 : (t + 1) * 16])
            nc.sync.dma_start(out[t * 128 : t * 128 + pt, :], oa[:pt, :])

    ctx.close()


def make_in_maps(meta, x, weights, n_cores):
    shard = meta["shard"]
    n_tiles = meta["n_tiles"]
    x_bf = np.ascontiguousarray(x.astype(BFNP))
    common = dict(
        x_bf=x_bf,
        wo1=np.ascontiguousarray(weights["wo1"].astype(BFNP)),
        wr1=np.ascontiguousarray(weights["wr1"].astype(BFNP)),
        wo2=np.ascontiguousarray(weights["wo2"].astype(BFNP)),
        wr2=np.ascontiguousarray(weights["wr2"].astype(BFNP)),
        b1=np.ascontiguousarray(weights["b1"].reshape(128, 1), dtype=np.float32),
        b2=np.ascontiguousarray(weights["b2"].reshape(1, 16).astype(BFNP)),
    )
    maps = []
    npad = n_tiles * 128 - shard
    for c in range(n_cores):
        m = dict(common)
        m.update(meta["per_core"][c])
        xo = np.pad(x[c * shard : (c + 1) * shard], ((0, npad), (0, 0)))
        m["x_own"] = np.ascontiguousarray(xo.astype(BFNP))
        m["x_ownT"] = np.ascontiguousarray(xo.T.astype(BFNP))
        maps.append(m)
    return maps


def trace_and_compile(meta, n_cores):
    nc = bacc.Bacc(
        "TRN2", target_bir_lowering=False, debug=False, num_devices=n_cores,
        num_swdge_queues=NQ,
    )
    with tile.TileContext(nc) as tc:
        build_kernel(nc, tc, meta, n_cores)
    nc.compile()
    return nc

# ---------------------------------------------------------------- entry point

N_CORES = 8
LAST_EXEC_TIME_NS = None
LAST_RESULTS = None


def kernel(x, train_pos_edge_index, W_out1, b_out1, W_root1, W_out2, b_out2, W_root2):
    """Full inputs in, full output out. Shards/compiles/runs on 8 TRN2 cores."""
    global LAST_EXEC_TIME_NS, LAST_RESULTS
    from concourse.bass_utils import run_bass_kernel_spmd

    x = np.ascontiguousarray(np.asarray(x), dtype=np.float32)
    meta = preprocess(x, np.asarray(train_pos_edge_index), N_CORES)
    nc = trace_and_compile(meta, N_CORES)
    weights = dict(
        wo1=np.asarray(W_out1), wr1=np.asarray(W_root1),
        wo2=np.asarray(W_out2), wr2=np.asarray(W_root2),
        b1=np.asarray(b_out1), b2=np.asarray(b_out2),
    )
    in_maps = make_in_maps(meta, x, weights, N_CORES)
    res = run_bass_kernel_spmd(nc, in_maps, core_ids=list(range(N_CORES)))
    LAST_RESULTS = res
    LAST_EXEC_TIME_NS = res.exec_time_ns
    out = np.concatenate([res.results[c]["out"] for c in range(N_CORES)], axis=0)
    return out.astype(np.float32)

